# revision 1
# baseline (speedup 1.0000x reference)
"""Trainium2 Bass kernel: 5x5 reflect-padded box-filter mean (LocalMean).

Full input:  image (32, 3, 512, 512) f32
Full output: same shape; out[r,c] = mean of the 5x5 window of the
reflect-padded image.

Strategy (pure data parallel over 8 NeuronCores, 4 images per core):
- Host pre-pads H and W by 2 with reflect -> (4, 3, 516, 516) per core.
- On-chip the filter is separable:
  * vertical 5-tap sum via TensorE banded matmuls (constant lower-band
    weight tile, 1/25-scaled; row blocks of 124 output rows so each
    block's 128 input rows live in a single SBUF tile -> one matmul),
  * horizontal 5-tap sum via one DVE reduce (window head) plus one DVE
    tensor_tensor_scan per block: H[j] = (V[j+4] + H[j-1]) - V[j-1].
- ScalarE copies the PSUM intermediate to SBUF (scan operands may not
  both live in PSUM); DMA (HBM ~13MB in + 12.6MB out per core) is the
  roofline bottleneck. GPSIMD is intentionally unused (2-input
  elementwise there is several times slower than DVE and contends for
  the DVE SBUF port).
"""

import numpy as np

N_CORES = 8
B, C, H, W = 32, 3, 512, 512
PB = B // N_CORES          # images per core
PAD = 2
HP, WP = H + 2 * PAD, W + 2 * PAD   # 516

# Output-row blocks of 124 (last 16): input rows [124b, 124b+128) per
# block all sit in one 128-partition tile, so the vertical matmul needs
# no cross-tile tail accumulation.
BLOCKS = [(0, 124), (124, 124), (248, 124), (372, 124), (496, 16)]

_CACHE = {}
# Experiment switches (default = the shipped configuration).
_CFG = {}


def _band_weights():
    # W[k, m] = 1/25 for 0 <= k-m <= 4: vertical 5-tap window starting at
    # output row m reads input rows m..m+4 of the padded block.
    def band(K, M):
        k = np.arange(K)[:, None]
        m = np.arange(M)[None, :]
        return (((k - m) >= 0) & ((k - m) <= 4)).astype(np.float32) / 25.0
    return band(128, 124), band(20, 16)


def _build(reps=1):
    import concourse.bacc as bacc
    import concourse.tile as tile
    from concourse import mybir

    f32 = mybir.dt.float32
    nc = bacc.Bacc("TRN2", target_bir_lowering=False, debug=False,
                   num_devices=N_CORES)
    x = nc.dram_tensor("x", [PB, C, HP, WP], f32, kind="ExternalInput").ap()
    wd = nc.dram_tensor("wd", [128, 124], f32, kind="ExternalInput").ap()
    wl = nc.dram_tensor("wl", [20, 16], f32, kind="ExternalInput").ap()
    y = nc.dram_tensor("y", [PB, C, H, W], f32, kind="ExternalOutput").ap()

    LOOKAHEAD = 3  # channel-images of input prefetched ahead of compute

    with tile.TileContext(nc) as tc:
        with (
            tc.tile_pool(name="wp", bufs=1) as wp,
            tc.tile_pool(name="xp", bufs=4 * (LOOKAHEAD + 2)) as xp,
            tc.tile_pool(name="xtp", bufs=LOOKAHEAD + 2) as xtp,
            tc.tile_pool(name="vp", bufs=4, space="PSUM") as vp,
            tc.tile_pool(name="vsp", bufs=6) as vsp,
            tc.tile_pool(name="op", bufs=8) as op,
        ):
            d_t = wp.tile([128, 124], f32)
            nc.sync.dma_start(d_t[:], wd[:, :])
            l_t = wp.tile([20, 16], f32)
            nc.sync.dma_start(l_t[:], wl[:, :])

            cis = [(n, c) for _ in range(reps)
                   for n in range(PB) for c in range(C)]
            loaded = {}  # step index -> list of 5 X tiles

            def load(s):
                n, c = cis[s]
                xts = []
                for b, (r0, h) in enumerate(BLOCKS):
                    kh = 128 if h == 124 else 20
                    pool = xp if kh == 128 else xtp
                    t = pool.tile([kh, WP], f32)
                    nc.sync.dma_start(t[:], x[n, c, r0:r0 + kh, :])
                    xts.append(t)
                loaded[s] = xts

            for s in range(min(LOOKAHEAD, len(cis))):
                load(s)

            for s, (n, c) in enumerate(cis):
                if s + LOOKAHEAD < len(cis):
                    load(s + LOOKAHEAD)
                xts = loaded.pop(s)

                for b, (r0, h) in enumerate(BLOCKS):
                    w_t = d_t if h == 124 else l_t
                    xt = xts[b]
                    v = vp.tile([128, WP], f32)
                    # V[m, :] = sum_{d=0..4} X[m+d, :] / 25, via banded
                    # matmul; N split at the PSUM bank boundary (fp32
                    # matmul N <= 512).
                    nc.tensor.matmul(v[0:h, 0:512], w_t[:], xt[:, 0:512],
                                     start=True, stop=True)
                    nc.tensor.matmul(v[0:h, 512:516], w_t[:], xt[:, 512:516],
                                     start=True, stop=True)

                    # PSUM -> SBUF once on the otherwise-idle ScalarE: DVE
                    # SBUF reads are 62 cycles/op cheaper than PSUM reads,
                    # so routing both scan operands through SBUF wins over
                    # reading V from PSUM directly (measured in sim).
                    vs = vsp.tile([128, WP], f32)
                    nc.scalar.copy(vs[0:h, :], v[0:h, :])

                    # Horizontal 5-tap sliding window on DVE:
                    #   H[0] = sum(Vs[0:5]);  H[j] = H[j-1] + Vs[j+4] - Vs[j-1]
                    o = op.tile([128, W], f32)
                    nc.vector.reduce_sum(o[0:h, 0:1], vs[0:h, 0:5],
                                         axis=mybir.AxisListType.X)
                    nc.vector.tensor_tensor_scan(
                        o[0:h, 1:512], vs[0:h, 5:516], vs[0:h, 0:511],
                        o[0:h, 0:1],
                        mybir.AluOpType.add, mybir.AluOpType.subtract)
                    # Output DMAs alternate between the two HWDGE queues:
                    # DMA *issue* costs ~0.65us per dma_start on an in-order
                    # sequencer, so issue work must be spread — SP carries
                    # the input DMAs, ACT the PSUM->SBUF copies, and each
                    # takes half the output issues to balance at ~58us.
                    dma_eng = nc.scalar if (s * 5 + b) % 2 == 0 else nc.sync
                    dma_eng.dma_start(y[n, c, r0:r0 + h, :], o[0:h, :])

    nc.compile()
    return nc


def _get_nc(reps=1):
    key = ("nc", reps)
    if key not in _CACHE:
        _CACHE[key] = _build(reps)
    return _CACHE[key]


def _shard_inputs(image: np.ndarray):
    image = np.ascontiguousarray(np.asarray(image, dtype=np.float32))
    padded = np.pad(image, ((0, 0), (0, 0), (PAD, PAD), (PAD, PAD)),
                    mode="reflect")
    d, dl = _band_weights()
    in_maps = []
    for i in range(N_CORES):
        in_maps.append({
            "x": np.ascontiguousarray(padded[i * PB:(i + 1) * PB]),
            "wd": d,
            "wl": dl,
        })
    return in_maps


def kernel(image: np.ndarray) -> np.ndarray:
    from concourse import bass_utils

    nc = _get_nc()
    in_maps = _shard_inputs(image)
    res = bass_utils.run_bass_kernel_spmd(nc, in_maps,
                                          core_ids=list(range(N_CORES)))
    return np.concatenate([res.results[i]["y"] for i in range(N_CORES)], axis=0)



# revision 9
# speedup vs baseline: 2.2059x; 2.2059x over previous
"""Trainium2 Bass kernel: 5x5 reflect-padded box-filter mean (LocalMean).

Full input:  image (32, 3, 512, 512) f32
Full output: same shape; out[r,c] = mean of the 5x5 window of the
reflect-padded image.

Strategy (pure data parallel over 8 NeuronCores, 4 images per core):
- Host reflect-pads, scales by 1/25 and converts to bf16, then lays the
  image out in a blocked format so each image is ONE contiguous DMA:
  xt[n, p, c, b, w] = padded[n, c, 124*b + p, w].  The 4-row overlaps
  between the 124-row output blocks are duplicated host-side.
- On chip, per image: one [128, 6192] bf16 load; vertical 5-tap sum via
  banded matmul (weights exactly 1.0, bf16, PSUM accumulates f32) in
  1024-column PSUM chunks; ScalarE copies PSUM -> SBUF bf16; horizontal
  5-tap as a 3-op add-tree fused across all 12 channel-blocks, with the
  column range split between DVE and Pool so neither engine exceeds the
  DMA roofline; junk at the 516-col block seams is skipped by a strided
  output DMA.  The 20 tail rows of all 4 images are packed into one
  [80, 1548] tile and processed with a block-diagonal band matmul.
- Host reassembles the blocked bf16 outputs and upcasts to f32.
  rel err ~9e-3 (bf16 rounding chain), under the 2e-2 gate.
"""

import numpy as np

N_CORES = 8
B, C, H, W = 32, 3, 512, 512
PB = B // N_CORES          # images per core
PAD = 2
HP, WP = H + 2 * PAD, W + 2 * PAD   # 516

NB = 4                      # big row blocks per image (124 out rows each)
BH = 124                    # out rows per big block
BW = WP                     # block pitch in V space (516)
OW = W                      # valid out cols per block (512)
FREE = C * NB * BW          # 6192 free cols in the fused per-image tile
TR = 20                     # tail input rows per image (496..516)
TO = 16                     # tail output rows per image (496..512)
TFREE = C * BW              # 1548
CHUNK = 1024                # PSUM chunk (f32, exactly 2 banks)

_CACHE = {}
# pool_cols: V columns of the horizontal add-tree handed to Pool (rest on
# DVE, split into dve_subranges so the tree starts before all PSUM->SBUF
# copies finish).  Pool adds run ~3.8x slower than DVE bf16 adds.
_CFG = {"pool_cols": 1292, "dve_subranges": 2, "lookahead": 3}


def _band(K, M, blocks=1):
    # W[k, m] = 1 for 0 <= k - m <= 4 within each diagonal block.
    kb, mb = K // blocks, M // blocks
    out = np.zeros((K, M), np.float32)
    for n in range(blocks):
        k = np.arange(kb)[:, None]
        m = np.arange(mb)[None, :]
        out[n * kb:(n + 1) * kb, n * mb:(n + 1) * mb] = (
            ((k - m) >= 0) & ((k - m) <= 4)).astype(np.float32)
    return out


def _build(reps=1):
    import concourse.bacc as bacc
    import concourse.tile as tile
    from concourse import mybir

    f32 = mybir.dt.float32
    bf16 = mybir.dt.bfloat16
    nc = bacc.Bacc("TRN2", target_bir_lowering=False, debug=False,
                   num_devices=N_CORES)
    xt = nc.dram_tensor("xt", [PB, 128, C, NB, BW], bf16,
                        kind="ExternalInput").ap()
    xtl = nc.dram_tensor("xtl", [PB * TR, C, BW], bf16,
                         kind="ExternalInput").ap()
    wd = nc.dram_tensor("wd", [128, BH], bf16, kind="ExternalInput").ap()
    wt = nc.dram_tensor("wt", [PB * TR, PB * TO], bf16,
                        kind="ExternalInput").ap()
    yt = nc.dram_tensor("yt", [PB, BH, C, NB, OW], bf16,
                        kind="ExternalOutput").ap()
    ytl = nc.dram_tensor("ytl", [PB * TO, C, OW], bf16,
                         kind="ExternalOutput").ap()

    LOOKAHEAD = _CFG["lookahead"]
    split = FREE - _CFG["pool_cols"]          # DVE gets V cols [0, split)
    nsub = _CFG["dve_subranges"]

    with tile.TileContext(nc) as tc:
        with (
            tc.tile_pool(name="wp", bufs=1) as wp,
            tc.tile_pool(name="xp", bufs=LOOKAHEAD + 1) as xp,
            tc.tile_pool(name="xtp", bufs=2) as xtp,
            tc.tile_pool(name="vp", bufs=2, space="PSUM") as vp,
            tc.tile_pool(name="tp", bufs=2, space="PSUM") as tp,
            tc.tile_pool(name="vsp", bufs=2) as vsp,
            tc.tile_pool(name="s2p", bufs=6) as s2p,
            tc.tile_pool(name="s4p", bufs=6) as s4p,
            tc.tile_pool(name="op", bufs=2) as op,
        ):
            d_t = wp.tile([128, BH], bf16)
            nc.sync.dma_start(d_t[:], wd[:, :])
            t_t = wp.tile([PB * TR, PB * TO], bf16)
            nc.sync.dma_start(t_t[:], wt[:, :])

            steps = [s for _ in range(reps) for s in [0, 1, 2, 3, "tail"]]
            loaded = {}

            def load(si):
                s = steps[si]
                if s == "tail":
                    t = xtp.tile([PB * TR, TFREE], bf16)
                    nc.sync.dma_start(
                        t[:], xtl[:, :, :].rearrange("p c w -> p (c w)"))
                else:
                    t = xp.tile([128, FREE], bf16)
                    nc.sync.dma_start(
                        t[:], xt[s].rearrange("p c b w -> p (c b w)"))
                loaded[si] = t

            for si in range(min(LOOKAHEAD, len(steps))):
                load(si)

            for si, s in enumerate(steps):
                if si + LOOKAHEAD < len(steps):
                    load(si + LOOKAHEAD)
                X = loaded.pop(si)

                if s == "tail":
                    ptn, w_t, free = PB * TO, t_t, TFREE
                    ranges = [("vector", 0, free)]
                else:
                    ptn, w_t, free = BH, d_t, FREE
                    bounds = [split * k // nsub for k in range(nsub)]
                    ranges = [("vector", b0, b1) for b0, b1 in
                              zip(bounds, bounds[1:] + [split])]
                    if split < free:
                        ranges.append(("gpsimd", split, free))

                # Vertical 5-tap sum: banded matmul into f32 PSUM chunks
                # (1024 f32 = 2 banks), ScalarE copies PSUM -> SBUF bf16.
                Vs = vsp.tile([ptn, free], bf16)
                nfull = free // CHUNK
                for k in range(nfull + (1 if free % CHUNK else 0)):
                    c0 = k * CHUNK
                    cw = min(CHUNK, free - c0)
                    pool = vp if (k < nfull and s != "tail") else tp
                    v = pool.tile([128, CHUNK], f32)
                    for m0 in range(0, cw, 512):
                        mw = min(512, cw - m0)
                        nc.tensor.matmul(v[0:ptn, m0:m0 + mw], w_t[:],
                                         X[:, c0 + m0:c0 + m0 + mw],
                                         start=True, stop=True)
                    nc.scalar.copy(Vs[:, c0:c0 + cw], v[0:ptn, 0:cw])

                # Horizontal 5-tap: 3-op add tree, column-split across
                # engines with private temps (local coords); junk at the
                # 516-col block seams is never read back.
                O = op.tile([ptn, free], bf16)
                for ename, c0, c1 in ranges:
                    eng = getattr(nc, ename)
                    L = min(c1 + 4, free) - c0
                    S2 = s2p.tile([ptn, L], bf16)
                    eng.tensor_add(S2[:, 0:L - 1], Vs[:, c0:c0 + L - 1],
                                   Vs[:, c0 + 1:c0 + L])
                    S4 = s4p.tile([ptn, L], bf16)
                    eng.tensor_add(S4[:, 0:L - 3], S2[:, 0:L - 3],
                                   S2[:, 2:L - 1])
                    eng.tensor_add(O[:, c0:c0 + L - 4], S4[:, 0:L - 4],
                                   Vs[:, c0 + 4:c0 + L])

                # Blocked output DMA (skips the 4 junk cols per block).
                # Issued from SP *after* this step's prefetch was issued.
                src = O[:].rearrange("p (cb w) -> p cb w", w=BW)[:, :, 0:OW]
                if s == "tail":
                    dst = ytl[:, :, :]
                else:
                    dst = yt[s].rearrange("p c b w -> p (c b) w")
                nc.sync.dma_start(dst, src)

    nc.compile()
    return nc


def _get_nc(reps=1):
    key = ("nc", reps)
    if key not in _CACHE:
        _CACHE[key] = _build(reps)
    return _CACHE[key]


def _shard_inputs(image: np.ndarray):
    import ml_dtypes
    bf16 = ml_dtypes.bfloat16

    image = np.asarray(image, dtype=np.float32)
    padded = np.pad(image * np.float32(1.0 / 25.0),
                    ((0, 0), (0, 0), (PAD, PAD), (PAD, PAD)),
                    mode="reflect")
    wd = _band(128, BH).astype(bf16)
    wt = _band(PB * TR, PB * TO, blocks=PB).astype(bf16)
    in_maps = []
    for i in range(N_CORES):
        p = padded[i * PB:(i + 1) * PB]            # [4, 3, 516, 516] f32
        # xt[n, p, c, b, w] = p[n, c, 124b + p, w]
        blocks = np.stack([p[:, :, BH * b:BH * b + 128, :]
                           for b in range(NB)], axis=2)  # [n, c, b, p, w]
        xt = np.ascontiguousarray(
            blocks.transpose(0, 3, 1, 2, 4)).astype(bf16)
        # xtl[(n r), c, w] = p[n, c, 496 + r, w]
        xtl = np.ascontiguousarray(
            p[:, :, HP - TR:, :].transpose(0, 2, 1, 3)
        ).reshape(PB * TR, C, WP).astype(bf16)
        in_maps.append({"xt": xt, "xtl": xtl, "wd": wd, "wt": wt})
    return in_maps


def kernel(image: np.ndarray) -> np.ndarray:
    from concourse import bass_utils

    nc = _get_nc()
    in_maps = _shard_inputs(image)
    res = bass_utils.run_bass_kernel_spmd(nc, in_maps,
                                          core_ids=list(range(N_CORES)))
    out = np.empty((B, C, H, W), np.float32)
    for i in range(N_CORES):
        yt = np.asarray(res.results[i]["yt"], dtype=np.float32)
        ytl = np.asarray(res.results[i]["ytl"], dtype=np.float32)
        # yt[n, p, c, b, w] -> rows 124b + p
        big = yt.transpose(0, 2, 3, 1, 4).reshape(PB, C, NB * BH, W)
        out[i * PB:(i + 1) * PB, :, 0:NB * BH, :] = big
        tl = ytl.reshape(PB, TO, C, W).transpose(0, 2, 1, 3)
        out[i * PB:(i + 1) * PB, :, NB * BH:, :] = tl
    return out


# revision 14
# speedup vs baseline: 2.2146x; 1.0039x over previous
"""Trainium2 Bass kernel: 5x5 reflect-padded box-filter mean (LocalMean).

Full input:  image (32, 3, 512, 512) f32
Full output: same shape; out[r,c] = mean of the 5x5 window of the
reflect-padded image.

Strategy (pure data parallel over 8 NeuronCores, 4 images per core):
- Host reflect-pads, scales by 1/25 and converts to bf16, then lays the
  image out in a blocked format so each image is ONE contiguous DMA:
  xt[n, p, c, b, w] = padded[n, c, 124*b + p, w].  The 4-row overlaps
  between the 124-row output blocks are duplicated host-side.
- On chip, per image: one [128, 6192] bf16 load; vertical 5-tap sum via
  banded matmul (weights exactly 1.0, bf16, PSUM accumulates f32) in
  1024-column PSUM chunks; ScalarE copies PSUM -> SBUF bf16; horizontal
  5-tap as a 3-op add-tree fused across all 12 channel-blocks, with the
  column range split between DVE and Pool so neither engine exceeds the
  DMA roofline; junk at the 516-col block seams is skipped by a strided
  output DMA.  The 20 tail rows of all 4 images are packed into one
  [80, 1548] tile and processed with a block-diagonal band matmul.
- Host reassembles the blocked bf16 outputs and upcasts to f32.
  rel err ~9e-3 (bf16 rounding chain), under the 2e-2 gate.
"""

import numpy as np

N_CORES = 8
B, C, H, W = 32, 3, 512, 512
PB = B // N_CORES          # images per core
PAD = 2
HP, WP = H + 2 * PAD, W + 2 * PAD   # 516

NB = 4                      # big row blocks per image (124 out rows each)
BH = 124                    # out rows per big block
BW = WP                     # block pitch in V space (516)
OW = W                      # valid out cols per block (512)
FREE = C * NB * BW          # 6192 free cols in the fused per-image tile
TR = 20                     # tail input rows per image (496..516)
TO = 16                     # tail output rows per image (496..512)
TFREE = C * BW              # 1548
CHUNK = 1024                # PSUM chunk (f32, exactly 2 banks)

_CACHE = {}
# pool_blocks: 516-col blocks of the horizontal add-tree handed to Pool
# (rest on DVE, split into dve_subranges so the tree starts before all
# PSUM->SBUF copies finish).  Pool adds run ~3.8x slower than DVE bf16
# adds.  Block-aligned so tree level 3 can write a packed output tile
# through a strided view (contiguous out-DMA, 12x fewer descriptors).
_CFG = {"pool_blocks": 2, "dve_subranges": 2, "lookahead": 3}


def _band(K, M, blocks=1):
    # W[k, m] = 1 for 0 <= k - m <= 4 within each diagonal block.
    kb, mb = K // blocks, M // blocks
    out = np.zeros((K, M), np.float32)
    for n in range(blocks):
        k = np.arange(kb)[:, None]
        m = np.arange(mb)[None, :]
        out[n * kb:(n + 1) * kb, n * mb:(n + 1) * mb] = (
            ((k - m) >= 0) & ((k - m) <= 4)).astype(np.float32)
    return out


def _build(reps=1):
    import concourse.bacc as bacc
    import concourse.tile as tile
    from concourse import mybir

    f32 = mybir.dt.float32
    bf16 = mybir.dt.bfloat16
    nc = bacc.Bacc("TRN2", target_bir_lowering=False, debug=False,
                   num_devices=N_CORES)
    xt = nc.dram_tensor("xt", [PB, 128, C, NB, BW], bf16,
                        kind="ExternalInput").ap()
    xtl = nc.dram_tensor("xtl", [PB * TR, C, BW], bf16,
                         kind="ExternalInput").ap()
    wd = nc.dram_tensor("wd", [128, BH], bf16, kind="ExternalInput").ap()
    wt = nc.dram_tensor("wt", [PB * TR, PB * TO], bf16,
                        kind="ExternalInput").ap()
    yt = nc.dram_tensor("yt", [PB, BH, C, NB, OW], bf16,
                        kind="ExternalOutput").ap()
    ytl = nc.dram_tensor("ytl", [PB * TO, C, OW], bf16,
                         kind="ExternalOutput").ap()

    LOOKAHEAD = _CFG["lookahead"]
    dve_blocks = C * NB - _CFG["pool_blocks"]  # DVE gets blocks [0, this)
    nsub = _CFG["dve_subranges"]

    with tile.TileContext(nc) as tc:
        with (
            tc.tile_pool(name="wp", bufs=1) as wp,
            tc.tile_pool(name="xp", bufs=LOOKAHEAD + 1) as xp,
            tc.tile_pool(name="xtp", bufs=2) as xtp,
            tc.tile_pool(name="vp", bufs=2, space="PSUM") as vp,
            tc.tile_pool(name="tp", bufs=2, space="PSUM") as tp,
            tc.tile_pool(name="vsp", bufs=2) as vsp,
            tc.tile_pool(name="s2p", bufs=6) as s2p,
            tc.tile_pool(name="s4p", bufs=6) as s4p,
            tc.tile_pool(name="op", bufs=2) as op,
        ):
            d_t = wp.tile([128, BH], bf16)
            nc.sync.dma_start(d_t[:], wd[:, :])
            t_t = wp.tile([PB * TR, PB * TO], bf16)
            nc.sync.dma_start(t_t[:], wt[:, :])

            steps = [s for _ in range(reps) for s in [0, 1, 2, 3, "tail"]]
            loaded = {}

            def load(si):
                s = steps[si]
                if s == "tail":
                    t = xtp.tile([PB * TR, TFREE], bf16)
                    nc.sync.dma_start(
                        t[:], xtl[:, :, :].rearrange("p c w -> p (c w)"))
                else:
                    t = xp.tile([128, FREE], bf16)
                    nc.sync.dma_start(
                        t[:], xt[s].rearrange("p c b w -> p (c b w)"))
                loaded[si] = t

            for si in range(min(LOOKAHEAD, len(steps))):
                load(si)

            for si, s in enumerate(steps):
                if si + LOOKAHEAD < len(steps):
                    load(si + LOOKAHEAD)
                X = loaded.pop(si)

                # ranges are (engine, b0, b1) in units of 516-col blocks
                if s == "tail":
                    ptn, w_t, free, nblk = PB * TO, t_t, TFREE, C
                    ranges = [("vector", 0, C)]
                else:
                    ptn, w_t, free, nblk = BH, d_t, FREE, C * NB
                    bounds = [dve_blocks * k // nsub for k in range(nsub)]
                    ranges = [("vector", b0, b1) for b0, b1 in
                              zip(bounds, bounds[1:] + [dve_blocks])]
                    if dve_blocks < nblk:
                        ranges.append(("gpsimd", dve_blocks, nblk))

                # Vertical 5-tap sum: banded matmul into f32 PSUM chunks
                # (1024 f32 = 2 banks), ScalarE copies PSUM -> SBUF bf16.
                # (+4 junk cols so block-aligned +4-offset views stay in
                # bounds; they are never read.)
                Vs = vsp.tile([ptn, free + 4], bf16)
                nfull = free // CHUNK
                for k in range(nfull + (1 if free % CHUNK else 0)):
                    c0 = k * CHUNK
                    cw = min(CHUNK, free - c0)
                    pool = vp if (k < nfull and s != "tail") else tp
                    v = pool.tile([128, CHUNK], f32)
                    for m0 in range(0, cw, 512):
                        mw = min(512, cw - m0)
                        nc.tensor.matmul(v[0:ptn, m0:m0 + mw], w_t[:],
                                         X[:, c0 + m0:c0 + m0 + mw],
                                         start=True, stop=True)
                    nc.scalar.copy(Vs[:, c0:c0 + cw], v[0:ptn, 0:cw])

                # Horizontal 5-tap: 3-op add tree, block-split across
                # engines with private temps (local coords); junk at the
                # 516-col block seams is never read back.  Level 3 writes
                # the packed O tile through strided per-block views so the
                # output DMA is fully contiguous.
                O = op.tile([ptn, nblk * OW], bf16)
                for ename, b0, b1 in ranges:
                    eng = getattr(nc, ename)
                    c0, nb = b0 * BW, b1 - b0
                    L = min(nb * BW + 4, free - c0)
                    S2 = s2p.tile([ptn, L], bf16)
                    eng.tensor_add(S2[:, 0:L - 1], Vs[:, c0:c0 + L - 1],
                                   Vs[:, c0 + 1:c0 + L])
                    S4 = s4p.tile([ptn, L], bf16)
                    eng.tensor_add(S4[:, 0:L - 3], S2[:, 0:L - 3],
                                   S2[:, 2:L - 1])
                    s4v = S4[:, 0:nb * BW].rearrange(
                        "p (b w) -> p b w", w=BW)[:, :, 0:OW]
                    vsv = Vs[:, c0 + 4:c0 + 4 + nb * BW].rearrange(
                        "p (b w) -> p b w", w=BW)[:, :, 0:OW]
                    ov = O[:, b0 * OW:b1 * OW].rearrange(
                        "p (b w) -> p b w", w=OW)
                    eng.tensor_add(ov, s4v, vsv)

                # Contiguous output DMA, issued from SP *after* this
                # step's prefetch was issued.
                dst = (ytl[:, :, :].rearrange("p c w -> p (c w)")
                       if s == "tail" else
                       yt[s].rearrange("p c b w -> p (c b w)"))
                nc.sync.dma_start(dst, O[:])

    nc.compile()
    return nc


def _get_nc(reps=1):
    key = ("nc", reps)
    if key not in _CACHE:
        _CACHE[key] = _build(reps)
    return _CACHE[key]


def _shard_inputs(image: np.ndarray):
    import ml_dtypes
    bf16 = ml_dtypes.bfloat16

    image = np.asarray(image, dtype=np.float32)
    padded = np.pad(image * np.float32(1.0 / 25.0),
                    ((0, 0), (0, 0), (PAD, PAD), (PAD, PAD)),
                    mode="reflect")
    wd = _band(128, BH).astype(bf16)
    wt = _band(PB * TR, PB * TO, blocks=PB).astype(bf16)
    in_maps = []
    for i in range(N_CORES):
        p = padded[i * PB:(i + 1) * PB]            # [4, 3, 516, 516] f32
        # xt[n, p, c, b, w] = p[n, c, 124b + p, w]
        blocks = np.stack([p[:, :, BH * b:BH * b + 128, :]
                           for b in range(NB)], axis=2)  # [n, c, b, p, w]
        xt = np.ascontiguousarray(
            blocks.transpose(0, 3, 1, 2, 4)).astype(bf16)
        # xtl[(n r), c, w] = p[n, c, 496 + r, w]
        xtl = np.ascontiguousarray(
            p[:, :, HP - TR:, :].transpose(0, 2, 1, 3)
        ).reshape(PB * TR, C, WP).astype(bf16)
        in_maps.append({"xt": xt, "xtl": xtl, "wd": wd, "wt": wt})
    return in_maps


def kernel(image: np.ndarray) -> np.ndarray:
    from concourse import bass_utils

    nc = _get_nc()
    in_maps = _shard_inputs(image)
    res = bass_utils.run_bass_kernel_spmd(nc, in_maps,
                                          core_ids=list(range(N_CORES)))
    out = np.empty((B, C, H, W), np.float32)
    for i in range(N_CORES):
        yt = np.asarray(res.results[i]["yt"], dtype=np.float32)
        ytl = np.asarray(res.results[i]["ytl"], dtype=np.float32)
        # yt[n, p, c, b, w] -> rows 124b + p
        big = yt.transpose(0, 2, 3, 1, 4).reshape(PB, C, NB * BH, W)
        out[i * PB:(i + 1) * PB, :, 0:NB * BH, :] = big
        tl = ytl.reshape(PB, TO, C, W).transpose(0, 2, 1, 3)
        out[i * PB:(i + 1) * PB, :, NB * BH:, :] = tl
    return out
